# revision 16
# baseline (speedup 1.0000x reference)
"""DeepSeek-V3 MoE gate (sigmoid + group-restricted top-k routing) on 8 TRN2
NeuronCores.

Strategy (data-parallel over tokens, per sharding hint):
  - x [16384, 7168] f32 is sharded 2048 tokens/core; weight [256, 7168] and
    bias [256] are replicated.
  - Per core: logits = x @ w.T computed on the PE at ~fp32 precision via an
    fp16 hi/lo 3-term split (x*2^8 = xh+xl, w*2^12 = wh+wl; logits*2^20 =
    xh@wh + xl@wh + xh@wl; the dropped xl@wl term is ~2^-24 relative).
    x is transposed on the PE (fp32, exact); the hi/lo split happens during
    the PSUM->SBUF eviction copies (ACT casts hi, DVE computes lo with one
    fused scalar_tensor_tensor). Sigmoid (ACT LUT, with the 2^-20 descale)
    then group-limited top-8 selection + weight normalization on the DVE via
    max8/max_index/match_replace; each tile's routing chain is interleaved
    into the next tile's GEMM so no engine sits idle.
  - Outputs (w [16384,8] f32, idx [16384,8] i32) are gathered host-side.
"""
import numpy as np

import concourse.bass as bass
import concourse.mybir as mybir
import concourse.tile as tile
from concourse import bacc
from concourse.bass_utils import run_bass_kernel_spmd

F32 = mybir.dt.float32
F16 = mybir.dt.float16
U32 = mybir.dt.uint32
I32 = mybir.dt.int32
AF = mybir.ActivationFunctionType
ALU = mybir.AluOpType
AX = mybir.AxisListType

N_CORES = 8
T = 16384
D = 7168
E = 256
TOPK = 8
N_GROUPS = 8
GSIZE = E // N_GROUPS       # 32
ROUTE_SCALE = 2.5

TPC = T // N_CORES          # 2048 tokens per core
ND = D // 128               # 56 contraction tiles
QUADS = ND // 4             # 14 (4 d-tiles share one PSUM staging bank)

XSCALE = 2.0 ** 8           # x pre-scale (keeps fp16 lo-parts normal)
WSCALE = 2.0 ** 12          # w pre-scale
DESCALE = 1.0 / (XSCALE * WSCALE)

NEG_MASK = -1.0e30          # "-inf" for group masking
MARK = -3.0                 # match_replace marker (outside score range)

_CACHE = {}


def _routing_thunks(nc, rt, scores, biasr, OW, OI, i):
    """Group-limited top-8 routing for one tile of 128 tokens, returned as a
    list of emission thunks so the caller can interleave them with the next
    tile's GEMM work on the same engines."""
    st = {}

    def t_s():
        st["s"] = rt.tile([128, E], F32, tag="s", name="s")
        nc.vector.tensor_add(st["s"][:], scores[:], biasr[:])

    def t_gmax1():
        st["gmax1"] = rt.tile([128, 8], F32, tag="gmax1", name="gmax1")
        nc.vector.tensor_reduce(
            st["gmax1"][:], st["s"].rearrange("p (g k) -> p g k", k=GSIZE),
            axis=AX.X, op=ALU.max,
        )

    def t_scr():
        st["scr"] = rt.tile([128, E], F32, tag="scr", name="scr")
        nc.vector.match_replace(
            st["scr"][:], in_to_replace=st["gmax1"][:], in_values=st["s"][:],
            imm_value=MARK,
        )

    def t_gsum():
        gmax2 = rt.tile([128, 8], F32, tag="gmax2", name="gmax2")
        nc.vector.tensor_reduce(
            gmax2[:], st["scr"].rearrange("p (g k) -> p g k", k=GSIZE),
            axis=AX.X, op=ALU.max,
        )
        st["gsum"] = rt.tile([128, 8], F32, tag="gsum", name="gsum")
        nc.vector.tensor_add(st["gsum"][:], st["gmax1"][:], gmax2[:])

    def t_pen():
        g8 = rt.tile([128, 8], F32, tag="g8", name="g8")
        nc.vector.max(g8[:], st["gsum"][:])
        st["pen"] = rt.tile([128, 8], F32, tag="pen", name="pen")
        nc.vector.tensor_scalar(
            st["pen"][:], st["gsum"][:], g8[:, 3:4], scalar2=NEG_MASK,
            op0=ALU.is_lt, op1=ALU.mult,
        )

    def t_masked():
        st["masked"] = rt.tile([128, E], F32, tag="masked", name="masked")
        pen3 = st["pen"].rearrange("p (g k) -> p g k", k=1).to_broadcast(
            [128, N_GROUPS, GSIZE]
        )
        nc.vector.tensor_tensor(
            st["masked"].rearrange("p (g k) -> p g k", k=GSIZE),
            st["s"].rearrange("p (g k) -> p g k", k=GSIZE), pen3, op=ALU.add,
        )

    def t_sel8():
        st["sel8"] = rt.tile([128, 8], F32, tag="sel8", name="sel8")
        nc.vector.max(st["sel8"][:], st["masked"][:])

    def t_idx8():
        st["idx8"] = rt.tile([128, 8], U32, tag="idx8", name="idx8")
        nc.vector.max_index(st["idx8"][:], st["sel8"][:], st["masked"][:])

    def t_scr2():
        st["scr2"] = rt.tile([128, E], F32, tag="scr2", name="scr2")
        nc.vector.match_replace(
            st["scr2"][:], in_to_replace=st["sel8"][:], in_values=st["masked"][:],
            imm_value=MARK,
        )

    def t_mark():
        st["mark"] = rt.tile([128, E], F32, tag="mark", name="mark")
        nc.vector.tensor_scalar(
            st["mark"][:], st["scr2"][:], MARK, scalar2=None, op0=ALU.is_equal
        )

    def t_dsc():
        st["dsc"] = rt.tile([128, E], F32, tag="dsc", name="dsc")
        nc.vector.tensor_tensor(st["dsc"][:], scores[:], st["mark"][:], op=ALU.mult)

    def t_ssel8():
        st["ssel8"] = rt.tile([128, 8], F32, tag="ssel8", name="ssel8")
        nc.vector.max(st["ssel8"][:], st["dsc"][:])

    def t_isel8():
        st["isel8"] = rt.tile([128, 8], U32, tag="isel8", name="isel8")
        nc.vector.max_index(st["isel8"][:], st["ssel8"][:], st["dsc"][:])

    def t_casts():
        st["idx8f"] = rt.tile([128, 8], F32, tag="idx8f", name="idx8f")
        nc.vector.tensor_copy(st["idx8f"][:], st["idx8"][:])
        st["isel8f"] = rt.tile([128, 8], F32, tag="isel8f", name="isel8f")
        nc.vector.tensor_copy(st["isel8f"][:], st["isel8"][:])

    def t_eq():
        st["eq"] = rt.tile([128, 8, 8], F32, tag="eq", name="eq")
        idx8_b = st["idx8f"].rearrange("p (j k) -> p j k", k=1).to_broadcast(
            [128, 8, 8]
        )
        isel8_b = st["isel8f"].rearrange("p (k j) -> p k j", k=1).to_broadcast(
            [128, 8, 8]
        )
        nc.vector.tensor_tensor(st["eq"][:], idx8_b, isel8_b, op=ALU.is_equal)

    def t_wj():
        prod = rt.tile([128, 8, 8], F32, tag="prod", name="prod")
        ssel8_b = st["ssel8"].rearrange("p (k j) -> p k j", k=1).to_broadcast(
            [128, 8, 8]
        )
        nc.vector.tensor_tensor(prod[:], st["eq"][:], ssel8_b, op=ALU.mult)
        st["wj"] = rt.tile([128, 8], F32, tag="wj", name="wj")
        nc.vector.tensor_reduce(st["wj"][:], prod[:], axis=AX.X, op=ALU.add)

    def t_rec():
        sumw = rt.tile([128, 1], F32, tag="sumw", name="sumw")
        nc.vector.tensor_reduce(sumw[:], st["wj"][:], axis=AX.X, op=ALU.add)
        st["rec"] = rt.tile([128, 1], F32, tag="rec", name="rec")
        nc.vector.reciprocal(st["rec"][:], sumw[:])

    def t_out():
        wout = rt.tile([128, TOPK], F32, tag="wout", name="wout")
        nc.vector.tensor_scalar(
            wout[:], st["wj"][:], st["rec"][:, 0:1], scalar2=ROUTE_SCALE,
            op0=ALU.mult, op1=ALU.mult,
        )
        iout = rt.tile([128, TOPK], I32, tag="iout", name="iout")
        nc.vector.tensor_copy(iout[:], st["idx8"][:])
        nc.sync.dma_start(OW[bass.ts(i, 128), :], wout[:])
        nc.sync.dma_start(OI[bass.ts(i, 128), :], iout[:])

    return [t_s, t_gmax1, t_scr, t_gsum, t_pen, t_masked, t_sel8, t_idx8,
            t_scr2, t_mark, t_dsc, t_ssel8, t_isel8, t_casts, t_eq, t_wj,
            t_rec, t_out]


def _build(tpc: int = TPC, fp16_split: bool = True):
    NT = tpc // 128
    nc = bacc.Bacc("TRN2", target_bir_lowering=False, debug=False)

    X = nc.dram_tensor("X", [tpc, D], F32, kind="ExternalInput")
    W = nc.dram_tensor("W", [E, D], F32, kind="ExternalInput")
    BIASR = nc.dram_tensor("BIASR", [128, E], F32, kind="ExternalInput")
    IDENT = nc.dram_tensor("IDENT", [128, 128], F32, kind="ExternalInput")
    OW = nc.dram_tensor("OW", [tpc, TOPK], F32, kind="ExternalOutput")
    OI = nc.dram_tensor("OI", [tpc, TOPK], I32, kind="ExternalOutput")

    with tile.TileContext(nc) as tc:
        with (
            tc.tile_pool(name="consts", bufs=1) as consts,
            tc.tile_pool(name="wtp", bufs=1) as wtp,
            tc.tile_pool(name="wnat", bufs=2) as wnat,
            tc.tile_pool(name="xin", bufs=2) as xin,
            tc.tile_pool(name="xtq", bufs=3) as xtq,
            tc.tile_pool(name="rt", bufs=2) as rt,
            tc.tile_pool(name="pst", bufs=5, space="PSUM") as pst,
            tc.tile_pool(name="psl", bufs=2, space="PSUM") as psl,
        ):
            ident = consts.tile([128, 128], F32)
            nc.sync.dma_start(ident[:], IDENT[:])
            biasr = consts.tile([128, E], F32)
            nc.sync.dma_start(biasr[:], BIASR[:])

            # x tile DMAs go on the HWDGE (sync) queue; W on SWDGE (gpsimd)
            # so the weight fetch doesn't serialize behind the first x tiles.
            xn_tiles = []
            for i in range(min(2, NT)):
                xn = xin.tile([128, D], F32, tag="xn", name="xn")
                if i == 0:
                    # split the first tile's fetch so the PE can start
                    # transposing after ~1/4 of the transfer
                    for h in range(4):
                        nc.sync.dma_start(
                            xn[:, bass.ts(h, D // 4)],
                            X[0:128, bass.ts(h, D // 4)],
                        )
                else:
                    nc.sync.dma_start(xn[:], X[bass.ts(i, 128), :])
                xn_tiles.append(xn)

            # ---- build transposed weight (resident in SBUF), fp16 hi/lo.
            # column block d holds (w*WSCALE).T[128d:128d+128, :]
            if fp16_split:
                wTh = wtp.tile([128, ND * E], F16)
                wTl = wtp.tile([128, ND * E], F16)
            else:
                wT = wtp.tile([128, ND * E], F32)
            for j in range(E // 128):  # 2 e-halves
                wn = wnat.tile([128, D], F32, tag="wn", name="wn")
                nc.gpsimd.dma_start(wn[:], W[bass.ts(j, 128), :])
                for q in range(QUADS):
                    stg = pst.tile([128, 512], F32, tag="stg", name="stg")
                    for k in range(4):
                        d = q * 4 + k
                        nc.tensor.transpose(
                            stg[:, bass.ts(k, 128)], wn[:, bass.ts(d, 128)], ident
                        )
                    src = stg.rearrange("p (k c) -> p k c", c=128)
                    sel = (slice(None), slice(q * 4, q * 4 + 4),
                           slice(j * 128, (j + 1) * 128))
                    if fp16_split:
                        dsth = wTh.rearrange("p (d c) -> p d c", c=E)[sel]
                        dstl = wTl.rearrange("p (d c) -> p d c", c=E)[sel]
                        nc.scalar.activation(dsth, src, AF.Copy, scale=WSCALE)
                        nc.vector.scalar_tensor_tensor(
                            dstl, src, WSCALE, dsth, op0=ALU.mult, op1=ALU.subtract
                        )
                    else:
                        dst = wT.rearrange("p (d c) -> p d c", c=E)[sel]
                        if q % 2 == 0:
                            nc.vector.tensor_copy(dst, src)
                        else:
                            nc.scalar.copy(dst, src)

            # ---- main loop over token tiles
            pending_routing = []  # thunks from the previous tile
            for i in range(NT):
                if i < 2:
                    xn = xn_tiles[i]
                else:
                    xn = xin.tile([128, D], F32, tag="xn", name="xn")
                    nc.sync.dma_start(xn[:], X[bass.ts(i, 128), :])

                logits = psl.tile([128, E], F32, tag="logits", name="logits")

                def emit_mms(q, tiles):
                    for k in range(4):
                        d = q * 4 + k
                        if fp16_split:
                            xqh, xql = tiles
                            nc.tensor.matmul(
                                logits[:], xqh[:, bass.ts(k, 128)],
                                wTh[:, bass.ds(d * E, E)],
                                start=(d == 0), stop=False,
                            )
                            nc.tensor.matmul(
                                logits[:], xql[:, bass.ts(k, 128)],
                                wTh[:, bass.ds(d * E, E)],
                                start=False, stop=False,
                            )
                            nc.tensor.matmul(
                                logits[:], xqh[:, bass.ts(k, 128)],
                                wTl[:, bass.ds(d * E, E)],
                                start=False, stop=(d == ND - 1 and k == 3),
                            )
                        else:
                            (xq,) = tiles
                            nc.tensor.matmul(
                                logits[:], xq[:, bass.ts(k, 128)],
                                wT[:, bass.ds(d * E, E)],
                                start=(d == 0), stop=(d == ND - 1),
                            )

                prev = None
                for q in range(QUADS):
                    stg = pst.tile([128, 512], F32, tag="stg", name="stg")
                    for k in range(4):
                        d = q * 4 + k
                        nc.tensor.transpose(
                            stg[:, bass.ts(k, 128)], xn[:, bass.ts(d, 128)], ident
                        )
                    if fp16_split:
                        xqh = xtq.tile([128, 512], F16, tag="xqh", name="xqh")
                        xql = xtq.tile([128, 512], F16, tag="xql", name="xql")
                        nc.scalar.activation(xqh[:], stg[:], AF.Copy, scale=XSCALE)
                        nc.vector.scalar_tensor_tensor(
                            xql[:], stg[:], XSCALE, xqh[:],
                            op0=ALU.mult, op1=ALU.subtract,
                        )
                        cur = (xqh, xql)
                    else:
                        xq = xtq.tile([128, 512], F32, tag="xq", name="xq")
                        if q % 2 == 0:
                            nc.vector.tensor_copy(xq[:], stg[:])
                        else:
                            nc.scalar.copy(xq[:], stg[:])
                        cur = (xq,)
                    # interleave the previous tile's routing chain (DVE) between
                    # this tile's extraction ops so extraction never queues
                    # behind the long serial routing chain
                    for _ in range(2):
                        if pending_routing:
                            pending_routing.pop(0)()
                    # software pipeline: previous quad's matmuls issue after this
                    # quad's transposes so the PE stays busy during the copies
                    if prev is not None:
                        emit_mms(q - 1, prev)
                    prev = cur
                emit_mms(QUADS - 1, prev)
                while pending_routing:
                    pending_routing.pop(0)()

                scores = rt.tile([128, E], F32, tag="scores", name="scores")
                nc.scalar.activation(
                    scores[:], logits[:], AF.Sigmoid,
                    scale=DESCALE if fp16_split else 1.0,
                )
                pending_routing = _routing_thunks(nc, rt, scores, biasr, OW, OI, i)

            while pending_routing:
                pending_routing.pop(0)()

    nc.compile()
    return nc


def kernel(x: np.ndarray, weight: np.ndarray, bias: np.ndarray):
    x = np.ascontiguousarray(x, dtype=np.float32)
    weight = np.ascontiguousarray(weight, dtype=np.float32)
    bias = np.ascontiguousarray(bias, dtype=np.float32)

    if "nc" not in _CACHE:
        _CACHE["nc"] = _build()
    nc = _CACHE["nc"]

    biasr = np.tile(bias[None, :], (128, 1))
    ident = np.eye(128, dtype=np.float32)
    in_maps = [
        {
            "X": x[c * TPC : (c + 1) * TPC],
            "W": weight,
            "BIASR": biasr,
            "IDENT": ident,
        }
        for c in range(N_CORES)
    ]
    global _last_in_maps
    _last_in_maps = in_maps
    res = run_bass_kernel_spmd(nc, in_maps, core_ids=list(range(N_CORES)))
    w = np.concatenate([r["OW"] for r in res.results], axis=0)
    idx = np.concatenate([r["OI"] for r in res.results], axis=0)
    return w, idx


_last_in_maps = None
